# revision 5
# baseline (speedup 1.0000x reference)
"""Trainium2 Bass kernel for nn_Decoder_33208687133135.

Reference computation (B=2048, D=64, L=64, H=512):
    z = swapaxes(koopman, 1, 2)                    # (B, D, L)
    s = MLP_s(z); t = MLP_t(z)                     # (B, D, D), 4 layers, tanh
    ds = diag(s); dt = diag(t)                     # (B, D)
    out = (x - dt) * exp(-ds)

Only the diagonal of the (B, D, D) MLP outputs is needed, so layer 4
reduces to a per-row dot product with a single W4 column.

v2 architecture ("ACT-wall buster").  The baseline was Scalar-engine
bound: 800 ACTIVATEs x ~674ns (96.7% busy) -- each [128,512] tanh pays a
~300ns fixed overhead.  Changes:

  * Super-blocks (SB) of 4 row-blocks; PSUM groups are [128, 4, 512]
    (one feature-chunk x 4 row-blocks = 4 banks), so each tanh ACTIVATE
    covers [128, 2048] with a constant per-partition bias.
  * L1 (K=64): two concurrent matmuls via PE row tiling (tile_position
    (0,0)/(64,0)); z ships in a partition-split layout, W1 duplicated
    into both partition halves.
  * L2 of the t-MLP runs fp8 e4m3 DoubleRow (K=256/pass, weights
    pre-scaled x16, descale folded into the ACT scale).  Only one
    MLP-layer runs fp8: sim rel-err 0.0138 vs gate 2e-2 (bf16: 0.0027).
  * L4: 8 accumulation chains (4 row-blocks x 2 MLPs) as M=1 matmuls,
    4 chains concurrent via 4-way column tiling.  L4 of SB s issues
    during SB s+1 (h3 double-buffered) so it never waits on the ACT
    tail of its own SB.
  * Finals batched at the end over [32, 512] (partition = block): one
    exp ACTIVATE + two DVE ops + one output DMA for the whole core.

Sharding: latent-parallel.  Core m handles latents i in [8m, 8m+8), all
2048 batches; weights replicated.  Per core: 32 blocks of 512 rows =
8 SBs; SB s <-> latent i0+s; block j = 4s + r, r = batch chunk.
"""

import numpy as np
import ml_dtypes

import concourse.mybir as mybir
import concourse.tile as tile
from concourse import bacc
from concourse.bass_utils import run_bass_kernel_spmd

BF16 = mybir.dt.bfloat16
FP8 = mybir.dt.float8e4
F32 = mybir.dt.float32
_bf = ml_dtypes.bfloat16
_f8 = ml_dtypes.float8_e4m3fn

B, D, L, H = 2048, 64, 64, 512
NCORES = 8
IPC = D // NCORES          # latent indices per core (8)
BN = 512                   # rows (batches) per block
BPI = B // BN              # blocks per latent index (4)
NBLK = IPC * BPI           # blocks per core (32)
NSB = IPC                  # super-blocks per core (8)
W8SCALE = 16.0             # fp8 weight pre-scale (w2t)

_CACHE = {}


def _build_nc():
    nc = bacc.Bacc("TRN2", target_bir_lowering=False, debug=False,
                   num_devices=NCORES)

    Tanh = mybir.ActivationFunctionType.Tanh
    Exp = mybir.ActivationFunctionType.Exp
    DR = mybir.MatmulPerfMode.DoubleRow

    # z split-partition layout [128, NSB*2*BN]:
    #   SB s, col block [s*1024 + rp*512 : +512]:
    #     partitions 0:64   = z rows (latent s, batch chunk 2*rp)
    #     partitions 64:128 = z rows (latent s, batch chunk 2*rp+1)
    z_d = nc.dram_tensor("z", [128, NSB * 2 * BN], BF16, kind="ExternalInput").ap()
    w1_d = nc.dram_tensor("w1", [2, 128, H], BF16, kind="ExternalInput").ap()
    w2s_d = nc.dram_tensor("w2s", [4, 128, H], BF16, kind="ExternalInput").ap()
    w2t_d = nc.dram_tensor("w2t", [2, 128, 2, H], FP8, kind="ExternalInput").ap()
    w3_d = nc.dram_tensor("w3", [2, 4, 128, H], BF16, kind="ExternalInput").ap()
    l4_d = nc.dram_tensor("l4", [2, 128, IPC, 4], BF16, kind="ExternalInput").ap()
    b123_d = nc.dram_tensor("b123", [2, 3, 128, 4], F32, kind="ExternalInput").ap()
    eb_d = nc.dram_tensor("eb", [NBLK, 1], F32, kind="ExternalInput").ap()
    xa_d = nc.dram_tensor("xa", [NBLK, BN], F32, kind="ExternalInput").ap()
    out_d = nc.dram_tensor("out", [NBLK, BN], F32, kind="ExternalOutput").ap()

    with tile.TileContext(nc) as tc:
        with (
            tc.tile_pool(name="const", bufs=1) as const,
            tc.tile_pool(name="h12", bufs=1) as h12,
            tc.tile_pool(name="h3p", bufs=2) as h3p,
            tc.tile_pool(name="scr", bufs=2) as scr,
            tc.tile_pool(name="fin", bufs=1) as fin,
            tc.tile_pool(name="pg", bufs=2, space="PSUM") as pg,
        ):
            # ---- constant tiles; DMA order: first-needed first ----
            w1_t = [const.tile([128, H], BF16, tag=f"w1_{mi}", name=f"w1_{mi}") for mi in range(2)]
            b_t = [[const.tile([128, 4], F32, tag=f"b_{mi}_{ly}", name=f"b_{mi}_{ly}")
                    for ly in range(3)] for mi in range(2)]
            zbig = const.tile([128, NSB * 2 * BN], BF16, tag="z")
            w2s_t = [const.tile([128, H], BF16, tag=f"w2s_{kc}", name=f"w2s_{kc}") for kc in range(4)]
            w2t_t = [const.tile([128, 2, H], FP8, tag=f"w2t_{pp}", name=f"w2t_{pp}") for pp in range(2)]
            w3_t = [[const.tile([128, H], BF16, tag=f"w3_{mi}_{kc}", name=f"w3_{mi}_{kc}")
                     for kc in range(4)] for mi in range(2)]
            l4_t = [const.tile([128, IPC, 4], BF16, tag=f"l4_{mi}", name=f"l4_{mi}") for mi in range(2)]
            eb_t = const.tile([NBLK, 1], F32, tag="eb")
            xa_t = const.tile([NBLK, BN], F32, tag="xa")

            nc.sync.dma_start(zbig[:, 0:BN], z_d[:, 0:BN])
            nc.sync.dma_start(w1_t[0][:], w1_d[0])
            for mi in range(2):
                for ly in range(3):
                    nc.sync.dma_start(b_t[mi][ly][:], b123_d[mi, ly])
            nc.sync.dma_start(w1_t[1][:], w1_d[1])
            nc.sync.dma_start(zbig[:, BN:2 * BN], z_d[:, BN:2 * BN])
            for kc in range(4):
                nc.sync.dma_start(w2s_t[kc][:], w2s_d[kc])
            for pp in range(2):
                nc.sync.dma_start(w2t_t[pp][:], w2t_d[pp])
            for mi in range(2):
                for kc in range(4):
                    nc.sync.dma_start(w3_t[mi][kc][:], w3_d[mi, kc])
            for mi in range(2):
                nc.sync.dma_start(l4_t[mi][:], l4_d[mi])
            nc.sync.dma_start(eb_t[:], eb_d)
            nc.sync.dma_start(xa_t[:], xa_d)
            for c in range(2, 2 * NSB):
                nc.sync.dma_start(zbig[:, c * BN:(c + 1) * BN],
                                  z_d[:, c * BN:(c + 1) * BN])

            ds_all = fin.tile([NBLK, BN], F32, tag="ds")
            dt_all = fin.tile([NBLK, BN], F32, tag="dt")

            def issue_l4(s, h3s, h3t, l4ps):
                """Layer 4 for SB s: 8 accumulation chains (r, mi), M=1,
                4-way column-tiled.  Chain (r, mi) -> psum partition
                32*(2*(r%2)+mi), free bank r//2."""
                for rpair in range(2):
                    for kc in range(4):
                        for r2 in range(2):
                            r = 2 * rpair + r2
                            for mi, h3 in ((0, h3s), (1, h3t)):
                                c = 32 * (2 * r2 + mi)
                                nc.tensor.matmul(
                                    l4ps[c:c + 1, rpair, :],
                                    l4_t[mi][:, s, kc:kc + 1],
                                    h3[:, r, kc, :],
                                    start=(kc == 0), stop=(kc == 3),
                                    tile_position=(0, c))

            def drain_l4(s, l4ps):
                sc = scr.tile([128, 2, BN], F32, tag="sc", name=f"sc_{s}")
                nc.vector.tensor_copy(sc[:], l4ps[:, 0:2, :])
                for r in range(4):
                    j = 4 * s + r
                    ps_, pt_ = 64 * (r % 2), 64 * (r % 2) + 32
                    nc.sync.dma_start(ds_all[j:j + 1, :],
                                      sc[ps_:ps_ + 1, r // 2, :])
                    nc.sync.dma_start(dt_all[j:j + 1, :],
                                      sc[pt_:pt_ + 1, r // 2, :])

            prev_l4 = [None]

            for s in range(NSB):
                h1s = h12.tile([128, 4, 4, BN], BF16, tag="h1s", name=f"h1s_{s}")
                h1t = h12.tile([128, 4, 2, 2, BN], FP8, tag="h1t", name=f"h1t_{s}")
                h2s = h12.tile([128, 4, 4, BN], BF16, tag="h2s", name=f"h2s_{s}")
                h2t = h12.tile([128, 4, 4, BN], BF16, tag="h2t", name=f"h2t_{s}")
                h3s = h3p.tile([128, 4, 4, BN], BF16, tag="h3s", name=f"h3s_{s}")
                h3t = h3p.tile([128, 4, 4, BN], BF16, tag="h3t", name=f"h3t_{s}")

                # ---- L1: row-tiled K=64 matmuls ----
                for mi, h1 in ((0, h1s), (1, h1t)):
                    for f in range(4):
                        p = pg.tile([128, 4, BN], F32, tag="g",
                                    name=f"p1_{s}_{mi}_{f}")
                        fs = slice(f * 128, (f + 1) * 128)
                        base = s * 2 * BN
                        for rp in range(2):
                            zlo = zbig[0:64, base + rp * BN:base + (rp + 1) * BN]
                            zhi = zbig[64:128, base + rp * BN:base + (rp + 1) * BN]
                            nc.tensor.matmul(p[:, 2 * rp, :],
                                             w1_t[mi][0:64, fs], zlo,
                                             start=True, stop=True,
                                             tile_position=(0, 0))
                            nc.tensor.matmul(p[:, 2 * rp + 1, :],
                                             w1_t[mi][64:128, fs], zhi,
                                             start=True, stop=True,
                                             tile_position=(64, 0))
                        bias = b_t[mi][0][:, f:f + 1]
                        if mi == 0:
                            nc.scalar.activation(h1s[:, :, f, :], p[:], Tanh,
                                                 bias=bias)
                        else:
                            nc.scalar.activation(h1t[:, :, f // 2, f % 2, :],
                                                 p[:], Tanh, bias=bias)

                # ---- L4 of previous SB ----
                if prev_l4[0] is not None:
                    ps_, s_, hs_, ht_ = prev_l4[0]
                    issue_l4(s_, hs_, ht_, ps_)
                    drain_l4(s_, ps_)
                    prev_l4[0] = None

                # ---- L2s: bf16, f-major, kc in ACT-arrival order ----
                for f in range(4):
                    p = pg.tile([128, 4, BN], F32, tag="g", name=f"p2s_{s}_{f}")
                    fs = slice(f * 128, (f + 1) * 128)
                    for kc in range(4):
                        for r in range(4):
                            nc.tensor.matmul(p[:, r, :], w2s_t[kc][:, fs],
                                             h1s[:, r, kc, :],
                                             start=(kc == 0), stop=(kc == 3))
                    nc.scalar.activation(h2s[:, :, f, :], p[:], Tanh,
                                         bias=b_t[0][1][:, f:f + 1])

                # ---- L2t: fp8 DoubleRow ----
                for f in range(4):
                    p = pg.tile([128, 4, BN], F32, tag="g", name=f"p2t_{s}_{f}")
                    fs = slice(f * 128, (f + 1) * 128)
                    for pp in range(2):
                        for r in range(4):
                            nc.tensor.matmul(p[:, r, :], w2t_t[pp][:, :, fs],
                                             h1t[:, r, pp, :, :],
                                             start=(pp == 0), stop=(pp == 1),
                                             perf_mode=DR)
                    nc.scalar.activation(h2t[:, :, f, :], p[:], Tanh,
                                         bias=b_t[1][1][:, f:f + 1],
                                         scale=1.0 / W8SCALE)

                # ---- L3: bf16 ----
                for mi, (h2, h3) in ((0, (h2s, h3s)), (1, (h2t, h3t))):
                    for f in range(4):
                        p = pg.tile([128, 4, BN], F32, tag="g",
                                    name=f"p3_{s}_{mi}_{f}")
                        fs = slice(f * 128, (f + 1) * 128)
                        for kc in range(4):
                            for r in range(4):
                                nc.tensor.matmul(p[:, r, :], w3_t[mi][kc][:, fs],
                                                 h2[:, r, kc, :],
                                                 start=(kc == 0), stop=(kc == 3))
                        nc.scalar.activation(h3[:, :, f, :], p[:], Tanh,
                                             bias=b_t[mi][2][:, f:f + 1])

                l4ps = pg.tile([128, 4, BN], F32, tag="g", name=f"l4p_{s}")
                prev_l4[0] = (l4ps, s, h3s, h3t)

            ps_, s_, hs_, ht_ = prev_l4[0]
            issue_l4(s_, hs_, ht_, ps_)
            drain_l4(s_, ps_)

            # ---- batched finals: out = (xa - dt) * exp(-ds + eb) ----
            es = fin.tile([NBLK, BN], F32, tag="es")
            nc.scalar.activation(es[:], ds_all[:], Exp, scale=-1.0, bias=eb_t[:])
            tmp = fin.tile([NBLK, BN], F32, tag="tmp")
            nc.vector.tensor_sub(tmp[:], xa_t[:], dt_all[:])
            outt = fin.tile([NBLK, BN], F32, tag="outt")
            nc.vector.tensor_mul(outt[:], tmp[:], es[:])
            nc.sync.dma_start(out_d[:], outt[:])

    nc.compile()
    return nc


def _prep_in_maps(inputs):
    """Host-side sharding: slice/cast per-core input arrays."""
    f32 = np.float32
    g = {k: np.asarray(v, f32) for k, v in inputs.items()}
    koopman, x = g["koopman"], g["x"]

    kt = np.ascontiguousarray(koopman.transpose(1, 2, 0)).astype(_bf)  # [L, D, B]
    xT = np.ascontiguousarray(x.T)  # [D, B]

    # w1 duplicated into both partition halves
    w1 = np.empty((2, 128, H), _bf)
    for mi, p in enumerate("st"):
        w1[mi, 0:64] = g[f"{p}W1"].astype(_bf)
        w1[mi, 64:128] = w1[mi, 0:64]

    # w2s: bf16 K-chunks; w2t: fp8 DoubleRow pairs, scaled
    w2s = np.ascontiguousarray(g["sW2"].reshape(4, 128, H)).astype(_bf)
    w2t_sc = np.clip(g["tW2"] * W8SCALE, -240, 240).astype(_f8)
    w2t = np.ascontiguousarray(w2t_sc.reshape(2, 2, 128, H).transpose(0, 2, 1, 3))

    w3 = np.stack([g["sW3"].reshape(4, 128, H),
                   g["tW3"].reshape(4, 128, H)]).astype(_bf)

    b123 = np.empty((2, 3, 128, 4), f32)
    for mi, p in enumerate("st"):
        for ly in range(3):
            b123[mi, ly] = g[f"{p}b{ly + 1}"].reshape(4, 128).T
    w4 = np.stack([g["sW4"], g["tW4"]])  # [2, H, D]
    b4s, b4t = g["sb4"], g["tb4"]

    in_maps = []
    for m in range(NCORES):
        i0 = m * IPC
        # z split-partition layout
        z = np.empty((128, NSB * 2 * BN), _bf)
        for s in range(NSB):
            col = kt[:, i0 + s, :]  # [L, B]
            for rp in range(2):
                lo = col[:, (2 * rp) * BN:(2 * rp + 1) * BN]
                hi = col[:, (2 * rp + 1) * BN:(2 * rp + 2) * BN]
                z[0:64, s * 2 * BN + rp * BN: s * 2 * BN + (rp + 1) * BN] = lo
                z[64:128, s * 2 * BN + rp * BN: s * 2 * BN + (rp + 1) * BN] = hi

        # l4: [2, 128, IPC, 4]: [mi, k, s, kc] = W4mi[kc*128+k, i0+s]
        l4 = np.ascontiguousarray(
            w4[:, :, i0:i0 + IPC].reshape(2, 4, 128, IPC).transpose(0, 2, 3, 1)
        ).astype(_bf)

        eb = np.repeat(-b4s[i0:i0 + IPC], BPI).astype(f32).reshape(NBLK, 1)
        xa = (xT[i0:i0 + IPC] - b4t[i0:i0 + IPC, None]).astype(f32)

        in_maps.append({
            "z": z, "w1": w1, "w2s": w2s, "w2t": w2t, "w3": w3, "l4": l4,
            "b123": b123, "eb": eb,
            "xa": np.ascontiguousarray(xa).reshape(NBLK, BN),
        })
    return in_maps


def _run(inputs, **run_kwargs):
    if "nc" not in _CACHE:
        _CACHE["nc"] = _build_nc()
    nc = _CACHE["nc"]
    in_maps = _prep_in_maps(inputs)
    res = run_bass_kernel_spmd(nc, in_maps, core_ids=list(range(NCORES)),
                               **run_kwargs)
    outT = np.empty((D, B), np.float32)
    for m in range(NCORES):
        i0 = m * IPC
        outT[i0:i0 + IPC] = np.asarray(
            res.results[m]["out"], np.float32).reshape(IPC, B)
    return np.ascontiguousarray(outT.T), res


def kernel(**inputs) -> np.ndarray:
    out, _ = _run(inputs)
    return out


# revision 7
# speedup vs baseline: 1.1846x; 1.1846x over previous
"""Trainium2 Bass kernel for nn_Decoder_33208687133135.

Reference computation (B=2048, D=64, L=64, H=512):
    z = swapaxes(koopman, 1, 2)                    # (B, D, L)
    s = MLP_s(z); t = MLP_t(z)                     # (B, D, D), 4 layers, tanh
    ds = diag(s); dt = diag(t)                     # (B, D)
    out = (x - dt) * exp(-ds)

Only the diagonal of the (B, D, D) MLP outputs is needed, so layer 4
reduces to a per-row dot product with a single W4 column.

v2 architecture ("ACT-wall buster").  The baseline was Scalar-engine
bound: 800 ACTIVATEs x ~674ns (96.7% busy) -- each [128,512] tanh pays a
~300ns fixed overhead.  Changes:

  * Super-blocks (SB) of 4 row-blocks; PSUM groups are [128, 4, 512]
    (one feature-chunk x 4 row-blocks = 4 banks), so each tanh ACTIVATE
    covers [128, 2048] with a constant per-partition bias.
  * L1 (K=64): two concurrent matmuls via PE row tiling (tile_position
    (0,0)/(64,0)); z ships in a partition-split layout, W1 duplicated
    into both partition halves.
  * L2 of the t-MLP runs fp8 e4m3 DoubleRow (K=256/pass, weights
    pre-scaled x16, descale folded into the ACT scale).  Only one
    MLP-layer runs fp8: sim rel-err 0.0138 vs gate 2e-2 (bf16: 0.0027).
  * L4: 8 accumulation chains (4 row-blocks x 2 MLPs) as M=1 matmuls,
    4 chains concurrent via 4-way column tiling.  L4 of SB s issues
    during SB s+1 (h3 double-buffered) so it never waits on the ACT
    tail of its own SB.
  * Finals batched at the end over [32, 512] (partition = block): one
    exp ACTIVATE + two DVE ops + one output DMA for the whole core.

Sharding: latent-parallel.  Core m handles latents i in [8m, 8m+8), all
2048 batches; weights replicated.  Per core: 32 blocks of 512 rows =
8 SBs; SB s <-> latent i0+s; block j = 4s + r, r = batch chunk.
"""

import numpy as np
import ml_dtypes

import concourse.mybir as mybir
import concourse.tile as tile
from concourse import bacc
from concourse.bass_utils import run_bass_kernel_spmd

BF16 = mybir.dt.bfloat16
FP8 = mybir.dt.float8e4
F32 = mybir.dt.float32
_bf = ml_dtypes.bfloat16
_f8 = ml_dtypes.float8_e4m3fn

B, D, L, H = 2048, 64, 64, 512
NCORES = 8
IPC = D // NCORES          # latent indices per core (8)
BN = 512                   # rows (batches) per block
BPI = B // BN              # blocks per latent index (4)
NBLK = IPC * BPI           # blocks per core (32)
NSB = IPC                  # super-blocks per core (8)
W8SCALE = 16.0             # fp8 weight pre-scale (w2t)

_CACHE = {}


def _build_nc():
    nc = bacc.Bacc("TRN2", target_bir_lowering=False, debug=False,
                   num_devices=NCORES)

    Tanh = mybir.ActivationFunctionType.Tanh
    Exp = mybir.ActivationFunctionType.Exp
    DR = mybir.MatmulPerfMode.DoubleRow

    # z split-partition layout [128, NSB*2*BN]:
    #   SB s, col block [s*1024 + rp*512 : +512]:
    #     partitions 0:64   = z rows (latent s, batch chunk 2*rp)
    #     partitions 64:128 = z rows (latent s, batch chunk 2*rp+1)
    z_d = nc.dram_tensor("z", [128, NSB * 2 * BN], BF16, kind="ExternalInput").ap()
    w1_d = nc.dram_tensor("w1", [2, 128, H], BF16, kind="ExternalInput").ap()
    w2s_d = nc.dram_tensor("w2s", [4, 128, H], BF16, kind="ExternalInput").ap()
    w2t_d = nc.dram_tensor("w2t", [2, 128, 2, H], FP8, kind="ExternalInput").ap()
    w3_d = nc.dram_tensor("w3", [2, 4, 128, H], BF16, kind="ExternalInput").ap()
    l4_d = nc.dram_tensor("l4", [2, 128, IPC, 4], BF16, kind="ExternalInput").ap()
    b123_d = nc.dram_tensor("b123", [2, 3, 128, 4], F32, kind="ExternalInput").ap()
    eb_d = nc.dram_tensor("eb", [NBLK, 1], F32, kind="ExternalInput").ap()
    xa_d = nc.dram_tensor("xa", [NBLK, BN], F32, kind="ExternalInput").ap()
    out_d = nc.dram_tensor("out", [NBLK, BN], F32, kind="ExternalOutput").ap()

    with tile.TileContext(nc) as tc:
        with (
            tc.tile_pool(name="const", bufs=1) as const,
            tc.tile_pool(name="h12", bufs=1) as h12,
            tc.tile_pool(name="h3p", bufs=2) as h3p,
            tc.tile_pool(name="scr", bufs=2) as scr,
            tc.tile_pool(name="fin", bufs=1) as fin,
            tc.tile_pool(name="pg", bufs=2, space="PSUM") as pg,
        ):
            # ---- constant tiles; DMA order: first-needed first ----
            w1_t = [const.tile([128, H], BF16, tag=f"w1_{mi}", name=f"w1_{mi}") for mi in range(2)]
            b_t = [[const.tile([128, 4], F32, tag=f"b_{mi}_{ly}", name=f"b_{mi}_{ly}")
                    for ly in range(3)] for mi in range(2)]
            zbig = const.tile([128, NSB * 2 * BN], BF16, tag="z")
            w2s_t = [const.tile([128, H], BF16, tag=f"w2s_{kc}", name=f"w2s_{kc}") for kc in range(4)]
            w2t_t = [const.tile([128, 2, H], FP8, tag=f"w2t_{pp}", name=f"w2t_{pp}") for pp in range(2)]
            w3_t = [[const.tile([128, H], BF16, tag=f"w3_{mi}_{kc}", name=f"w3_{mi}_{kc}")
                     for kc in range(4)] for mi in range(2)]
            l4_t = [const.tile([128, IPC, 4], BF16, tag=f"l4_{mi}", name=f"l4_{mi}") for mi in range(2)]
            eb_t = const.tile([NBLK, 1], F32, tag="eb")
            xa_t = const.tile([NBLK, BN], F32, tag="xa")

            nc.sync.dma_start(zbig[:, 0:BN], z_d[:, 0:BN])
            nc.sync.dma_start(w1_t[0][:], w1_d[0])
            for mi in range(2):
                for ly in range(3):
                    nc.sync.dma_start(b_t[mi][ly][:], b123_d[mi, ly])
            nc.sync.dma_start(w1_t[1][:], w1_d[1])
            nc.sync.dma_start(zbig[:, BN:2 * BN], z_d[:, BN:2 * BN])
            for kc in range(4):
                nc.sync.dma_start(w2s_t[kc][:], w2s_d[kc])
            for pp in range(2):
                nc.sync.dma_start(w2t_t[pp][:], w2t_d[pp])
            for mi in range(2):
                for kc in range(4):
                    nc.sync.dma_start(w3_t[mi][kc][:], w3_d[mi, kc])
            for mi in range(2):
                nc.sync.dma_start(l4_t[mi][:], l4_d[mi])
            nc.sync.dma_start(eb_t[:], eb_d)
            nc.sync.dma_start(xa_t[:], xa_d)
            for c in range(2, 2 * NSB):
                nc.sync.dma_start(zbig[:, c * BN:(c + 1) * BN],
                                  z_d[:, c * BN:(c + 1) * BN])

            ds_all = fin.tile([NBLK, BN], F32, tag="ds")
            dt_all = fin.tile([NBLK, BN], F32, tag="dt")

            def issue_l4(s, h3s, h3t, l4ps):
                """Layer 4 for SB s: 8 accumulation chains (r, mi), M=1,
                4-way column-tiled.  Chain (r, mi) -> psum partition
                32*(2*(r%2)+mi), free bank r//2."""
                for rpair in range(2):
                    for kc in range(4):
                        for r2 in range(2):
                            r = 2 * rpair + r2
                            for mi, h3 in ((0, h3s), (1, h3t)):
                                c = 32 * (2 * r2 + mi)
                                nc.tensor.matmul(
                                    l4ps[c:c + 1, rpair, :],
                                    l4_t[mi][:, s, kc:kc + 1],
                                    h3[:, r, kc, :],
                                    start=(kc == 0), stop=(kc == 3),
                                    tile_position=(0, c))

            def drain_l4(s, l4ps):
                sc = scr.tile([128, 2, BN], F32, tag="sc", name=f"sc_{s}")
                nc.vector.tensor_copy(sc[:], l4ps[:, 0:2, :])
                for r in range(4):
                    j = 4 * s + r
                    ps_, pt_ = 64 * (r % 2), 64 * (r % 2) + 32
                    nc.sync.dma_start(ds_all[j:j + 1, :],
                                      sc[ps_:ps_ + 1, r // 2, :])
                    nc.sync.dma_start(dt_all[j:j + 1, :],
                                      sc[pt_:pt_ + 1, r // 2, :])

            prev_l4 = [None]

            def make_h(s):
                return dict(
                    h1s=h12.tile([128, 4, 4, BN], BF16, tag="h1s", name=f"h1s_{s}"),
                    h1t=h12.tile([128, 4, 2, 2, BN], FP8, tag="h1t", name=f"h1t_{s}"),
                    h2s=h12.tile([128, 4, 4, BN], BF16, tag="h2s", name=f"h2s_{s}"),
                    h2t=h12.tile([128, 4, 4, BN], BF16, tag="h2t", name=f"h2t_{s}"),
                    h3s=h3p.tile([128, 4, 4, BN], BF16, tag="h3s", name=f"h3s_{s}"),
                    h3t=h3p.tile([128, 4, 4, BN], BF16, tag="h3t", name=f"h3t_{s}"),
                )

            def emit_l1_group(s, mi, f, h):
                """One L1 psum group: 4 row-tiled K=64 matmuls + fused tanh."""
                p = pg.tile([128, 4, BN], F32, tag="g", name=f"p1_{s}_{mi}_{f}")
                fs = slice(f * 128, (f + 1) * 128)
                base = s * 2 * BN
                for rp in range(2):
                    zlo = zbig[0:64, base + rp * BN:base + (rp + 1) * BN]
                    zhi = zbig[64:128, base + rp * BN:base + (rp + 1) * BN]
                    nc.tensor.matmul(p[:, 2 * rp, :], w1_t[mi][0:64, fs], zlo,
                                     start=True, stop=True, tile_position=(0, 0))
                    nc.tensor.matmul(p[:, 2 * rp + 1, :], w1_t[mi][64:128, fs],
                                     zhi, start=True, stop=True,
                                     tile_position=(64, 0))
                bias = b_t[mi][0][:, f:f + 1]
                if mi == 0:
                    nc.scalar.activation(h["h1s"][:, :, f, :], p[:], Tanh,
                                         bias=bias)
                else:
                    nc.scalar.activation(h["h1t"][:, :, f // 2, f % 2, :], p[:],
                                         Tanh, bias=bias)

            def emit_l2s(s, h):
                for f in range(4):
                    p = pg.tile([128, 4, BN], F32, tag="g", name=f"p2s_{s}_{f}")
                    fs = slice(f * 128, (f + 1) * 128)
                    for kc in range(4):
                        for r in range(4):
                            nc.tensor.matmul(p[:, r, :], w2s_t[kc][:, fs],
                                             h["h1s"][:, r, kc, :],
                                             start=(kc == 0), stop=(kc == 3))
                    nc.scalar.activation(h["h2s"][:, :, f, :], p[:], Tanh,
                                         bias=b_t[0][1][:, f:f + 1])

            def emit_l2t(s, h):
                for f in range(4):
                    p = pg.tile([128, 4, BN], F32, tag="g", name=f"p2t_{s}_{f}")
                    fs = slice(f * 128, (f + 1) * 128)
                    for pp in range(2):
                        for r in range(4):
                            nc.tensor.matmul(p[:, r, :], w2t_t[pp][:, :, fs],
                                             h["h1t"][:, r, pp, :, :],
                                             start=(pp == 0), stop=(pp == 1),
                                             perf_mode=DR)
                    nc.scalar.activation(h["h2t"][:, :, f, :], p[:], Tanh,
                                         bias=b_t[1][1][:, f:f + 1],
                                         scale=1.0 / W8SCALE)

            def emit_l3_group(s, mi, f, h):
                h2 = h["h2s"] if mi == 0 else h["h2t"]
                h3 = h["h3s"] if mi == 0 else h["h3t"]
                p = pg.tile([128, 4, BN], F32, tag="g", name=f"p3_{s}_{mi}_{f}")
                fs = slice(f * 128, (f + 1) * 128)
                for kc in range(4):
                    for r in range(4):
                        nc.tensor.matmul(p[:, r, :], w3_t[mi][kc][:, fs],
                                         h2[:, r, kc, :],
                                         start=(kc == 0), stop=(kc == 3))
                nc.scalar.activation(h3[:, :, f, :], p[:], Tanh,
                                     bias=b_t[mi][2][:, f:f + 1])

            # ---- software-pipelined emission ----
            # cycle s: L2s(s), L2t(s), L4(s-1), then L3(s) f-groups with
            # L1(s+1) groups interleaved (keeps tiled-MM stretches short and
            # the ACT chain for L1(s+1) finishes during L3(s) PE work).
            hcur = make_h(0)
            for mi in range(2):
                for f in range(4):
                    emit_l1_group(0, mi, f, hcur)
            for s in range(NSB):
                hnext = make_h(s + 1) if s + 1 < NSB else None
                emit_l2s(s, hcur)
                emit_l2t(s, hcur)
                if prev_l4[0] is not None:
                    ps_, s_, hs_, ht_ = prev_l4[0]
                    issue_l4(s_, hs_, ht_, ps_)
                    drain_l4(s_, ps_)
                    prev_l4[0] = None
                for gi in range(8):
                    mi, f = gi // 4, gi % 4
                    emit_l3_group(s, mi, f, hcur)
                    if hnext is not None:
                        emit_l1_group(s + 1, mi, f, hnext)
                l4ps = pg.tile([128, 4, BN], F32, tag="g", name=f"l4p_{s}")
                prev_l4[0] = (l4ps, s, hcur["h3s"], hcur["h3t"])
                hcur = hnext

            ps_, s_, hs_, ht_ = prev_l4[0]
            issue_l4(s_, hs_, ht_, ps_)
            drain_l4(s_, ps_)

            # ---- batched finals: out = (xa - dt) * exp(-ds + eb) ----
            es = fin.tile([NBLK, BN], F32, tag="es")
            nc.scalar.activation(es[:], ds_all[:], Exp, scale=-1.0, bias=eb_t[:])
            tmp = fin.tile([NBLK, BN], F32, tag="tmp")
            nc.vector.tensor_sub(tmp[:], xa_t[:], dt_all[:])
            outt = fin.tile([NBLK, BN], F32, tag="outt")
            nc.vector.tensor_mul(outt[:], tmp[:], es[:])
            nc.sync.dma_start(out_d[:], outt[:])

    nc.compile()
    return nc


def _prep_in_maps(inputs):
    """Host-side sharding: slice/cast per-core input arrays."""
    f32 = np.float32
    g = {k: np.asarray(v, f32) for k, v in inputs.items()}
    koopman, x = g["koopman"], g["x"]

    kt = np.ascontiguousarray(koopman.transpose(1, 2, 0)).astype(_bf)  # [L, D, B]
    xT = np.ascontiguousarray(x.T)  # [D, B]

    # w1 duplicated into both partition halves
    w1 = np.empty((2, 128, H), _bf)
    for mi, p in enumerate("st"):
        w1[mi, 0:64] = g[f"{p}W1"].astype(_bf)
        w1[mi, 64:128] = w1[mi, 0:64]

    # w2s: bf16 K-chunks; w2t: fp8 DoubleRow pairs, scaled
    w2s = np.ascontiguousarray(g["sW2"].reshape(4, 128, H)).astype(_bf)
    w2t_sc = np.clip(g["tW2"] * W8SCALE, -240, 240).astype(_f8)
    w2t = np.ascontiguousarray(w2t_sc.reshape(2, 2, 128, H).transpose(0, 2, 1, 3))

    w3 = np.stack([g["sW3"].reshape(4, 128, H),
                   g["tW3"].reshape(4, 128, H)]).astype(_bf)

    b123 = np.empty((2, 3, 128, 4), f32)
    for mi, p in enumerate("st"):
        for ly in range(3):
            b123[mi, ly] = g[f"{p}b{ly + 1}"].reshape(4, 128).T
    w4 = np.stack([g["sW4"], g["tW4"]])  # [2, H, D]
    b4s, b4t = g["sb4"], g["tb4"]

    in_maps = []
    for m in range(NCORES):
        i0 = m * IPC
        # z split-partition layout
        z = np.empty((128, NSB * 2 * BN), _bf)
        for s in range(NSB):
            col = kt[:, i0 + s, :]  # [L, B]
            for rp in range(2):
                lo = col[:, (2 * rp) * BN:(2 * rp + 1) * BN]
                hi = col[:, (2 * rp + 1) * BN:(2 * rp + 2) * BN]
                z[0:64, s * 2 * BN + rp * BN: s * 2 * BN + (rp + 1) * BN] = lo
                z[64:128, s * 2 * BN + rp * BN: s * 2 * BN + (rp + 1) * BN] = hi

        # l4: [2, 128, IPC, 4]: [mi, k, s, kc] = W4mi[kc*128+k, i0+s]
        l4 = np.ascontiguousarray(
            w4[:, :, i0:i0 + IPC].reshape(2, 4, 128, IPC).transpose(0, 2, 3, 1)
        ).astype(_bf)

        eb = np.repeat(-b4s[i0:i0 + IPC], BPI).astype(f32).reshape(NBLK, 1)
        xa = (xT[i0:i0 + IPC] - b4t[i0:i0 + IPC, None]).astype(f32)

        in_maps.append({
            "z": z, "w1": w1, "w2s": w2s, "w2t": w2t, "w3": w3, "l4": l4,
            "b123": b123, "eb": eb,
            "xa": np.ascontiguousarray(xa).reshape(NBLK, BN),
        })
    return in_maps


def _run(inputs, **run_kwargs):
    if "nc" not in _CACHE:
        _CACHE["nc"] = _build_nc()
    nc = _CACHE["nc"]
    in_maps = _prep_in_maps(inputs)
    res = run_bass_kernel_spmd(nc, in_maps, core_ids=list(range(NCORES)),
                               **run_kwargs)
    outT = np.empty((D, B), np.float32)
    for m in range(NCORES):
        i0 = m * IPC
        outT[i0:i0 + IPC] = np.asarray(
            res.results[m]["out"], np.float32).reshape(IPC, B)
    return np.ascontiguousarray(outT.T), res


def kernel(**inputs) -> np.ndarray:
    out, _ = _run(inputs)
    return out
